# revision 15
# baseline (speedup 1.0000x reference)
"""APPNP (MLP + 10 sparse propagation iterations) on 8 Trainium2 NeuronCores.

Design (source-sharded; all FLOPs on device, host does indexing only):
  - Nodes sharded by id: core c owns nodes [c*12500, (c+1)*12500) as BOTH
    source shard (z' rows it gathers from) and dest shard (the 98 local
    blocks it combines after the ReduceScatter). Per-core local slot
    layout from a serpentine in-degree bin-pack (98 blocks x 128 slots).
  - Edges partitioned by SOURCE core. Each core gathers its edges' source
    rows from its OWN z' shard only (no all-gather): z' lives bf16 in
    256B-strided padded rows ([12544, 128] bf16, data in cols 0:64) so the
    SWDGE gather uses 128B-payload descriptors (the 256B-alignment assert
    in bass.dma_gather is a transpose-only hardware restriction; the
    instruction is emitted directly with elem_size=64/elem_step=128).
  - Scatter-add over the GLOBAL dest space (784 blocks = 8 cores x 98) as
    one-hot selection-matrix matmuls into per-supergroup PSUM f32
    accumulators (S built on-device in bf16: DVE is_equal runs in 4x mode;
    a configurable fraction on ScalarE as Abs/Relu pairs). Chunk schedule
    shared across cores via a max-over-cores K table; chunks stream in
    <=63-chunk gather instructions consumed in emission order.
  - Per iteration ONE ReduceScatter(add) reduces the f32 partial
    aggregations [128, 784, 64] (partition-major for contiguous 1.8KB
    write descriptors; the collective reads a transposed AP so each core
    receives its own 98 blocks) into zr [12544, 64] f32.
  - Combine (2 fused DVE ops per block): z' = 0.9*dinv^2*agg + 0.1*dinv*h
    (bf16, into padded zp rows); last iteration z = 0.9*dinv*agg + 0.1*h
    written f32 to zout.
  - MLP (h = relu(x@W0+b0)@W1+b1) runs once on-device in bf16 from a
    host-transposed x shard; precomputes ahd=0.1*dinv*h and ahL=0.1*h.
"""

import os
import numpy as np
import ml_dtypes

import concourse.bass as bass
import concourse.bacc as bacc
import concourse.tile as tile
import concourse.mybir as mybir
from concourse.bass_utils import run_bass_kernel_spmd

F32 = mybir.dt.float32
BF16 = mybir.dt.bfloat16
I16 = mybir.dt.int16
NPBF16 = ml_dtypes.bfloat16

N = 100000
F_IN = 512
H = 64
NCORES = 8
ALPHA = 0.1
NITER = int(os.environ.get("APPNP_NITER", "10"))
SKIP = os.environ.get("APPNP_SKIP", "")
ACT_FRAC10 = int(os.environ.get("APPNP_ACT10", "1"))
POOL_FRAC10 = int(os.environ.get("APPNP_POOL10", "1"))
PF32 = bool(os.environ.get("APPNP_PF32", "1"))  # f32 partials+ReduceScatter

DPC = N // NCORES          # 12500 real nodes per core
NBLK = 98                  # local blocks of 128 dest slots
SLOTS = NBLK * 128         # 12544 padded slots per core
GBLK = NCORES * NBLK       # 784 global dest blocks
SGB = 7                    # blocks per supergroup
NSG = GBLK // SGB          # 112 supergroups (global)
NTOT = NCORES * SLOTS      # 100352 global dest slots
GMAX = 63                  # chunks per dma_gather instruction
ZPAD = 128                 # padded z' row width (256B bf16 stride)

PDT = BF16
NPPDT = NPBF16


def _prep_graph(edge_index, edge_weight):
    """Host-side: shard/sort/pad edges; returns per-core data + shared K."""
    row = edge_index[0].astype(np.int64)
    col = edge_index[1].astype(np.int64)
    loops = np.arange(N, dtype=np.int64)
    row = np.concatenate([row, loops])
    col = np.concatenate([col, loops])
    w = np.concatenate([edge_weight.astype(np.float32), np.ones(N, np.float32)])

    # degrees exactly as the reference: deg = segment_sum(w, row)
    deg = np.bincount(row, weights=w.astype(np.float64), minlength=N)
    deg = deg.astype(np.float32)
    dinv = np.where(deg > 0, 1.0 / np.sqrt(np.maximum(deg, 1e-30)), 0.0).astype(
        np.float32
    )

    perm = _make_perm(row, col)
    return _prep_graph2(row, col, w, dinv, perm)


def _make_perm(row, col):
    """slot = perm[core][local_old].

    The chunk schedule pads each (srccore, block) edge count to the
    max-over-cores ceil(cnt/128), so pack each dest core's 12500 nodes
    into its 98 blocks minimizing sum_b max_a ceil(cnt_ab/128): greedy
    over nodes in decreasing max-component in-degree, assigning to the
    bin with the smallest (new K, new max count).
    """
    csrc = row // DPC
    dcnt = np.bincount(col * NCORES + csrc, minlength=N * NCORES).reshape(
        N, NCORES
    )  # per-node in-degree split by source core (incl self-loop)
    perm = np.empty((NCORES, DPC), dtype=np.int64)
    for c in range(NCORES):
        deg = dcnt[c * DPC : (c + 1) * DPC].astype(np.int64)  # [DPC, 8]
        order = np.argsort(-deg.max(axis=1), kind="stable")
        loads = np.zeros((NBLK, NCORES), dtype=np.int64)
        fill = np.zeros(NBLK, dtype=np.int64)
        rank = np.empty(DPC, dtype=np.int64)
        binof = np.empty(DPC, dtype=np.int64)
        for i in order:
            nm = (loads + deg[i]).max(axis=1)
            score = ((nm + 127) >> 7) * 100000 + nm
            score[fill >= 128] = 1 << 60
            b = int(np.argmin(score))
            binof[i] = b
            rank[i] = fill[b]
            fill[b] += 1
            loads[b] += deg[i]
        perm[c] = binof * 128 + rank
    return perm


def _prep_graph2(row, col, w, dinv, perm):
    csrc = row // DPC
    sidx_all = perm[csrc, row - csrc * DPC]  # gather idx in own shard
    assert sidx_all.max() < 32768

    cdst = col // DPC
    ldst = perm[cdst, col - cdst * DPC]
    gb = cdst * NBLK + ldst // 128  # global dest block
    prt = ldst % 128

    # per-(srccore, globalblock) counts -> shared K table
    key = csrc * GBLK + gb
    cnt = np.bincount(key, minlength=NCORES * GBLK).reshape(NCORES, GBLK)
    K = np.maximum(1, (cnt.max(axis=0) + 127) // 128).astype(np.int64)  # [GBLK]

    chunk_off = np.zeros(GBLK, dtype=np.int64)
    chunk_off[1:] = np.cumsum(K)[:-1]
    totch = int(K.sum())
    nslots = totch * 128

    # gather instructions: flat split of the chunk stream
    instr_C = []
    left = totch
    while left > 0:
        c = min(GMAX, left)
        instr_C.append(c)
        left -= c
    instr_C = np.array(instr_C, dtype=np.int64)

    # per-core slot arrays
    per_core = []
    for c in range(NCORES):
        m = csrc == c
        eb, ep, esi, ew = gb[m], prt[m], sidx_all[m], w[m]
        order = np.lexsort((ep, eb))
        eb, ep, esi, ew = eb[order], ep[order], esi[order], ew[order]
        gstart = np.searchsorted(eb, np.arange(GBLK))
        rank = np.arange(len(eb)) - gstart[eb]
        slots = chunk_off[eb] * 128 + rank

        sidx = np.zeros(nslots, dtype=np.int16)   # gather index (pad -> 0)
        sdst = np.full(nslots, 999.0, dtype=np.float32)  # S value (pad -> 999)
        sw = np.zeros(nslots, dtype=np.float32)
        sidx[slots] = esi.astype(np.int16)
        sdst[slots] = ep.astype(np.float32)
        sw[slots] = ew
        per_core.append((sidx, sdst, sw))

    allones = bool(np.all(w == 1.0))
    return dinv, K, chunk_off, instr_C, totch, per_core, perm, allones


def _pack_gidx(sidx, instr_C):
    """Pack int16 gather indices into [16, totch*8] (SWDGE wrap layout).

    Index i of instruction j (chunk offset coff) lands at
    [i%16 + 16*k, coff*8 + i//16] for k in 0..8.
    """
    totch = len(sidx) // 128
    out = np.zeros((16, totch * 8), dtype=np.int16)
    pos = 0
    coff = 0
    for c in instr_C:
        c = int(c)
        n = c * 128
        vals = sidx[pos : pos + n]
        i = np.arange(n)
        out[i % 16, coff * 8 + (i // 16)] = vals
        pos += n
        coff += c
    assert pos == len(sidx)
    return np.tile(out, (8, 1))


def _raw_gather(nc, out_ap, in_ap, idxs_ap, num_idxs, elem_size, stride_bytes):
    """Emit InstDMAGatherAnt directly: the bass helper's 256B elem-size
    assert is a transpose-mode hardware restriction; non-transpose SWDGE
    gathers take byte-granular payloads (mirrored by the executor)."""
    g = nc.gpsimd
    _in_ap = g.lower_ap_dma(in_ap, for_custom_bir_dma=True)
    return g.add_instruction(
        mybir.InstDMAGatherAnt(
            name=g.bass.get_next_instruction_name(),
            ins=[
                *_in_ap,
                g.lower_ap(idxs_ap),
                g.lower_val_access(g.to_reg(num_idxs)),
            ],
            outs=[g.lower_ap(out_ap)],
            transpose=False,
            num_idxs=num_idxs,
            elem_size=elem_size,
            stride_bytes_256=stride_bytes // 256,
            gen_mode=0,
            single_packet=False,
            queue_num=0,
            sbuf_tokens_per_rank=0,
            sbuf_free_dim_per_rank=0,
            sbuf_free_dim_pad_per_rank=0,
            sbuf_byte_offset=0,
        )
    )


def _build_program(K, chunk_off, instr_C, totch, allones=True):
    """Build the SPMD bass program (same for all cores)."""
    nc = bacc.Bacc("TRN2", target_bir_lowering=False, debug=False, num_devices=NCORES)
    RDT = F32 if PF32 else PDT  # partials / ReduceScatter dtype

    # ---- I/O ----
    xT = nc.dram_tensor("xT", [F_IN, SLOTS], PDT, kind="ExternalInput")
    W0c = nc.dram_tensor("W0c", [4, 128, H], PDT, kind="ExternalInput")
    W1 = nc.dram_tensor("W1", [H, H], PDT, kind="ExternalInput")
    b0c = nc.dram_tensor("b0c", [H, 1], F32, kind="ExternalInput")
    b1r = nc.dram_tensor("b1r", [128, H], F32, kind="ExternalInput")
    # coef rows: 0=c2 (0.9*dinv^2), 1=c2L (0.9*dinv), 2=a1 (0.1*dinv), 3=dinv
    coef = nc.dram_tensor("coef", [128, 4, NBLK], F32, kind="ExternalInput")
    iota_d = nc.dram_tensor("iota", [128, 128], PDT, kind="ExternalInput")
    gidx_d = nc.dram_tensor("gidx", [128, totch * 8], I16, kind="ExternalInput")
    destv_d = nc.dram_tensor("destv", [128, totch], F32, kind="ExternalInput")
    ndestv_d = nc.dram_tensor("ndestv", [128, totch], F32, kind="ExternalInput")
    if not allones:
        wv_d = nc.dram_tensor("wv", [128, totch], F32, kind="ExternalInput")
    zout = nc.dram_tensor("zout", [SLOTS, H], F32, kind="ExternalOutput")

    # internal DRAM (double buffered): padded z' shard, partial aggs, RS out
    zp = [nc.dram_tensor(f"zp{i}", [SLOTS, ZPAD], PDT) for i in range(2)]
    part = [nc.dram_tensor(f"part{i}", [128, GBLK, H], RDT) for i in range(2)]
    zr = [nc.dram_tensor(f"zr{i}", [SLOTS, H], RDT) for i in range(2)]

    n_instr = len(instr_C)
    # chunk -> (instr, local offset)
    ch2gi = np.zeros(totch, dtype=np.int64)
    ch2lc = np.zeros(totch, dtype=np.int64)
    instr_coff = np.zeros(n_instr, dtype=np.int64)
    pos = 0
    for gi, c in enumerate(instr_C):
        instr_coff[gi] = pos
        ch2gi[pos : pos + c] = gi
        ch2lc[pos : pos + c] = np.arange(c)
        pos += int(c)

    with tile.TileContext(nc) as tc:
        with (
            tc.tile_pool(name="res", bufs=1) as res,
            tc.tile_pool(name="msg", bufs=4) as msgp,
            tc.tile_pool(name="sp", bufs=12) as sp,
            tc.tile_pool(name="outp", bufs=4) as outp,
            tc.tile_pool(name="psum", bufs=4, space="PSUM") as psp,
        ):
            # ---- residents ----
            iota_sb = res.tile([128, 128], PDT)
            nc.sync.dma_start(out=iota_sb[:], in_=iota_d[:])
            ndestv_sb = res.tile([128, totch], F32)
            nc.sync.dma_start(out=ndestv_sb[:], in_=ndestv_d[:])
            if not allones:
                wv_sb = res.tile([128, totch], F32)
                nc.sync.dma_start(out=wv_sb[:], in_=wv_d[:])
            destv_sb = res.tile([128, totch], F32)
            nc.sync.dma_start(out=destv_sb[:], in_=destv_d[:])
            coef_sb = res.tile([128, 4, NBLK], F32)
            nc.sync.dma_start(out=coef_sb[:], in_=coef[:])
            c2_sb = coef_sb[:, 0, :]
            c2L_sb = coef_sb[:, 1, :]
            a1_sb = coef_sb[:, 2, :]
            dinv_sb = coef_sb[:, 3, :]
            ahd_sb = res.tile([128, NBLK, H], PDT)  # 0.1*dinv*h
            ahL_sb = res.tile([128, NBLK, H], PDT)  # 0.1*h
            w0_sb = res.tile([128, 4, H], PDT)
            nc.sync.dma_start(out=w0_sb[:], in_=W0c.ap().rearrange("k p h -> p k h"))
            w1_sb = res.tile([H, H], PDT)
            nc.sync.dma_start(out=w1_sb[:], in_=W1[:])
            b0_sb = res.tile([H, 1], F32)
            nc.sync.dma_start(out=b0_sb[:], in_=b0c[:])
            b1_sb = res.tile([128, H], F32)
            nc.sync.dma_start(out=b1_sb[:], in_=b1r[:])

            # ---- MLP: h = relu(x@W0+b0)@W1 + b1; z'_0 = dinv*h into zp0;
            # ahd = 0.1*dinv*h, ahL = 0.1*h kept resident ----
            xT_r = xT.ap().rearrange("(k p) c -> p k c", p=128)  # [128,4,SLOTS]
            zp0_r = zp[0].ap().rearrange("(b p) c -> p b c", p=128)
            with (
                tc.tile_pool(name="mlp", bufs=3) as mlp,
                tc.tile_pool(name="mpsum", bufs=2, space="PSUM") as mpsum,
            ):
                for msg_ in range(NBLK // SGB):
                    zslab = outp.tile(
                        [128, SGB, H], F32 if NITER == 0 else PDT, tag="zslab0"
                    )
                    for j in range(SGB):
                        b = msg_ * SGB + j
                        xt = mlp.tile([128, 4, 128], PDT, tag="xt")
                        nc.sync.dma_start(
                            out=xt[:], in_=xT_r[:, :, b * 128 : (b + 1) * 128]
                        )
                        ph1 = mpsum.tile([H, 128], F32, tag="ph1")
                        for k in range(4):
                            nc.tensor.matmul(
                                ph1[:],
                                w0_sb[:, k, :],
                                xt[:, k, :],
                                start=(k == 0),
                                stop=(k == 3),
                            )
                        h1T = mlp.tile([H, 128], PDT, tag="h1T")
                        nc.scalar.activation(
                            h1T[:],
                            ph1[:],
                            mybir.ActivationFunctionType.Relu,
                            bias=b0_sb[:, 0:1],
                        )
                        ph2 = mpsum.tile([128, H], F32, tag="ph2")
                        nc.tensor.matmul(ph2[:], h1T[:], w1_sb[:], start=True, stop=True)
                        ht = mlp.tile([128, H], F32, tag="ht")
                        nc.vector.tensor_tensor(
                            ht[:], ph2[:], b1_sb[:], mybir.AluOpType.add
                        )
                        nc.vector.tensor_scalar_mul(
                            ahd_sb[:, b, :], ht[:], a1_sb[:, b : b + 1]
                        )
                        nc.vector.tensor_scalar_mul(ahL_sb[:, b, :], ht[:], ALPHA)
                        nc.vector.tensor_scalar_mul(
                            zslab[:, j, :], ht[:], dinv_sb[:, b : b + 1]
                        )
                    if NITER == 0:
                        nc.sync.dma_start(
                            out=zout.ap().rearrange("(b p) h -> p b h", p=128)[
                                :, msg_ * SGB : (msg_ + 1) * SGB, :
                            ],
                            in_=zslab[:],
                        )
                    else:
                        nc.sync.dma_start(
                            out=zp0_r[:, msg_ * SGB : (msg_ + 1) * SGB, 0:H],
                            in_=zslab[:],
                        )

            # ---- propagation iterations ----
            for it in range(NITER):
                last = it == NITER - 1
                zsrc = zp[it % 2].ap()[:, 0:H]  # 256B-strided bf16 rows
                tiles = {}  # gi -> msg tile

                def chunk_mt(t, tiles=tiles, zsrc=zsrc):
                    gi = int(ch2gi[t])
                    if gi not in tiles:
                        C = int(instr_C[gi])
                        coff = int(instr_coff[gi])
                        gx = sp.tile([128, GMAX * 8], I16, tag="gx", bufs=3)
                        nc.sync.dma_start(
                            out=gx[:, : C * 8],
                            in_=gidx_d[:, coff * 8 : (coff + C) * 8],
                        )
                        mt = msgp.tile([128, GMAX, H], PDT, tag="msg")
                        if SKIP != "gather":
                            _raw_gather(
                                nc,
                                mt[:, :C, :],
                                zsrc,
                                gx[:, : C * 8],
                                C * 128,
                                H,
                                ZPAD * 2,
                            )
                        else:
                            nc.vector.memset(mt[:, 0:1, :], 0.0)
                        tiles[gi] = mt
                    return tiles[gi][:, int(ch2lc[t]), :]

                for sg in range(NSG):
                    acc = psp.tile([128, SGB * H], F32, name="acc", tag="acc")
                    for j in range(SGB):
                        if SKIP == "mm":
                            break
                        b = sg * SGB + j
                        a = acc[:, j * H : (j + 1) * H]
                        kb = int(K[b])
                        for ck in range(kb):
                            t = int(chunk_off[b]) + ck
                            mtv = chunk_mt(t)
                            if not allones:
                                nc.vector.tensor_scalar_mul(
                                    mtv, mtv, wv_sb[:, t : t + 1]
                                )
                            st = sp.tile([128, 128], PDT, tag="S")
                            r10 = t % 10
                            if r10 < ACT_FRAC10:
                                nc.scalar.activation(
                                    st[:],
                                    iota_sb[:],
                                    mybir.ActivationFunctionType.Abs,
                                    bias=ndestv_sb[:, t : t + 1],
                                )
                                nc.scalar.activation(
                                    st[:],
                                    st[:],
                                    mybir.ActivationFunctionType.Relu,
                                    bias=1.0,
                                    scale=-1.0,
                                )
                            else:
                                seng = (
                                    nc.gpsimd
                                    if r10 < ACT_FRAC10 + POOL_FRAC10
                                    else nc.vector
                                )
                                seng.tensor_scalar(
                                    st[:],
                                    iota_sb[:],
                                    destv_sb[:, t : t + 1],
                                    None,
                                    mybir.AluOpType.is_equal,
                                )
                            nc.tensor.matmul(
                                a,
                                st[:],
                                mtv,
                                start=(ck == 0),
                                stop=(ck == kb - 1),
                            )
                    # drain supergroup PSUM -> partials (one copy + DMA)
                    pslab = outp.tile([128, SGB * H], RDT, tag="pslab")
                    if SKIP == "mm":
                        nc.vector.memset(pslab[:, 0:1], 0.0)
                    else:
                        nc.vector.tensor_copy(pslab[:], acc[:])
                    nc.sync.dma_start(
                        out=part[it % 2].ap()[:, sg * SGB : (sg + 1) * SGB, :],
                        in_=pslab[:].rearrange("p (b h) -> p b h", h=H),
                    )

                # ReduceScatter over the global partials; core c receives
                # its own 98 blocks (transposed AP: (b p) iteration order)
                nc.gpsimd.collective_compute(
                    "ReduceScatter",
                    mybir.AluOpType.add,
                    replica_groups=[list(range(NCORES))],
                    ins=[part[it % 2].ap().rearrange("p b h -> b p h").opt()],
                    outs=[zr[it % 2].ap().opt()],
                )

                # combine: z' = c2*agg + ahd (bf16) / last: z = c2L*agg + ahL
                zr_r = zr[it % 2].ap().rearrange("(b p) h -> p b h", p=128)
                cmul = c2L_sb if last else c2_sb
                ah = ahL_sb if last else ahd_sb
                zdst_r = (
                    zout.ap().rearrange("(b p) h -> p b h", p=128)
                    if last
                    else zp[(it + 1) % 2].ap().rearrange("(b p) c -> p b c", p=128)
                )
                for cg in range(NBLK // SGB):
                    zrt = outp.tile([128, SGB, H], RDT, tag="zrt")
                    nc.sync.dma_start(
                        out=zrt[:], in_=zr_r[:, cg * SGB : (cg + 1) * SGB, :]
                    )
                    zslab = outp.tile(
                        [128, SGB, H], F32 if last else PDT,
                        tag="zslabL" if last else "zslab",
                    )
                    for j in range(SGB):
                        b = cg * SGB + j
                        tmp = outp.tile([128, H], F32, tag="ctmp")
                        nc.vector.tensor_scalar_mul(
                            tmp[:], zrt[:, j, :], cmul[:, b : b + 1]
                        )
                        nc.vector.tensor_tensor(
                            zslab[:, j, :], tmp[:], ah[:, b, :],
                            mybir.AluOpType.add,
                        )
                    if last:
                        nc.sync.dma_start(
                            out=zdst_r[:, cg * SGB : (cg + 1) * SGB, :],
                            in_=zslab[:],
                        )
                    else:
                        nc.sync.dma_start(
                            out=zdst_r[:, cg * SGB : (cg + 1) * SGB, 0:H],
                            in_=zslab[:],
                        )

    nc.compile()
    return nc


def kernel(x, edge_index, edge_weight, W0, b0, W1, b1):
    x = np.asarray(x, dtype=np.float32)
    dinv, K, chunk_off, instr_C, totch, per_core, perm, allones = _prep_graph(
        np.asarray(edge_index), np.asarray(edge_weight)
    )

    in_maps = []
    for c in range(NCORES):
        sidx, sdst, sw = per_core[c]
        g = _pack_gidx(sidx, instr_C)

        destv = sdst.reshape(totch, 128).T.copy()  # [128, totch]

        xs = np.zeros((SLOTS, F_IN), dtype=np.float32)
        xs[perm[c]] = x[c * DPC : (c + 1) * DPC]
        xT = np.ascontiguousarray(xs.T).astype(NPPDT)  # [F_IN, SLOTS]

        dv = np.zeros(SLOTS, dtype=np.float32)
        dv[perm[c]] = dinv[c * DPC : (c + 1) * DPC]
        dv2 = dv.reshape(NBLK, 128).T  # [128, NBLK]
        coef = np.ascontiguousarray(
            np.stack(
                [
                    (1.0 - ALPHA) * dv2 * dv2,  # c2
                    (1.0 - ALPHA) * dv2,        # c2L
                    ALPHA * dv2,                # a1
                    dv2,                        # dinv
                ]
            ).transpose(1, 0, 2)
        ).astype(np.float32)

        in_maps.append(
            {
                "xT": xT,
                "W0c": np.asarray(W0, np.float32).reshape(4, 128, H).astype(NPPDT),
                "W1": np.asarray(W1, np.float32).astype(NPPDT),
                "b0c": np.asarray(b0, np.float32).reshape(H, 1).copy(),
                "b1r": np.broadcast_to(
                    np.asarray(b1, np.float32), (128, H)
                ).copy(),
                "coef": coef,
                "iota": np.broadcast_to(
                    np.arange(128, dtype=np.float32), (128, 128)
                ).astype(NPPDT),
                "gidx": g,
                "destv": destv,
                "ndestv": -destv,
                **(
                    {}
                    if allones
                    else {"wv": sw.reshape(totch, 128).T.copy()}
                ),
            }
        )

    nc = _build_program(K, chunk_off, instr_C, totch, allones)
    res = run_bass_kernel_spmd(nc, in_maps, core_ids=list(range(NCORES)))

    global LAST_PERM, LAST_NC
    LAST_PERM = perm
    LAST_NC = nc
    out = np.empty((N, H), dtype=np.float32)
    for c in range(NCORES):
        out[c * DPC : (c + 1) * DPC] = res.results[c]["zout"][perm[c]]
    return out


# revision 18
# speedup vs baseline: 1.2087x; 1.2087x over previous
"""APPNP (MLP + 10 sparse propagation iterations) on 8 Trainium2 NeuronCores.

Design (source-sharded; all FLOPs on device, host does indexing only):
  - Nodes sharded by id: core c owns nodes [c*12500, (c+1)*12500) as BOTH
    source shard (z' rows it gathers from) and dest shard (the 98 local
    blocks it combines after the ReduceScatter). Per-core local slot
    layout from a serpentine in-degree bin-pack (98 blocks x 128 slots).
  - Edges partitioned by SOURCE core. Each core gathers its edges' source
    rows from its OWN z' shard only (no all-gather): z' lives bf16 in
    256B-strided padded rows ([12544, 128] bf16, data in cols 0:64) so the
    SWDGE gather uses 128B-payload descriptors (the 256B-alignment assert
    in bass.dma_gather is a transpose-only hardware restriction; the
    instruction is emitted directly with elem_size=64/elem_step=128).
  - Scatter-add over the GLOBAL dest space (784 blocks = 8 cores x 98) as
    one-hot selection-matrix matmuls into per-supergroup PSUM f32
    accumulators (S built on-device in bf16: DVE is_equal runs in 4x mode;
    a configurable fraction on ScalarE as Abs/Relu pairs). Chunk schedule
    shared across cores via a max-over-cores K table; chunks stream in
    <=63-chunk gather instructions consumed in emission order.
  - Per iteration ONE ReduceScatter(add) reduces the f32 partial
    aggregations [128, 784, 64] (partition-major for contiguous 1.8KB
    write descriptors; the collective reads a transposed AP so each core
    receives its own 98 blocks) into zr [12544, 64] f32.
  - Combine (2 fused DVE ops per block): z' = 0.9*dinv^2*agg + 0.1*dinv*h
    (bf16, into padded zp rows); last iteration z = 0.9*dinv*agg + 0.1*h
    written f32 to zout.
  - MLP (h = relu(x@W0+b0)@W1+b1) runs once on-device in bf16 from a
    host-transposed x shard; precomputes ahd=0.1*dinv*h and ahL=0.1*h.
"""

import os
import numpy as np
import ml_dtypes

import concourse.bass as bass
import concourse.bacc as bacc
import concourse.tile as tile
import concourse.mybir as mybir
from concourse.bass_utils import run_bass_kernel_spmd

F32 = mybir.dt.float32
BF16 = mybir.dt.bfloat16
I16 = mybir.dt.int16
NPBF16 = ml_dtypes.bfloat16

N = 100000
F_IN = 512
H = 64
NCORES = 8
ALPHA = 0.1
NITER = int(os.environ.get("APPNP_NITER", "10"))
SKIP = os.environ.get("APPNP_SKIP", "")
ACT_FRAC10 = int(os.environ.get("APPNP_ACT10", "1"))
POOL_FRAC10 = int(os.environ.get("APPNP_POOL10", "0"))
PF32 = bool(os.environ.get("APPNP_PF32", "1"))  # f32 partials+ReduceScatter

DPC = N // NCORES          # 12500 real nodes per core
NBLK = 98                  # local blocks of 128 dest slots
SLOTS = NBLK * 128         # 12544 padded slots per core
GBLK = NCORES * NBLK       # 784 global dest blocks
SGB = 7                    # blocks per supergroup
NSG = GBLK // SGB          # 112 supergroups (global)
NTOT = NCORES * SLOTS      # 100352 global dest slots
GMAX = 63                  # chunks per dma_gather instruction
ZPAD = 128                 # padded z' row width (256B bf16 stride)

PDT = BF16
NPPDT = NPBF16


def _prep_graph(edge_index, edge_weight):
    """Host-side: shard/sort/pad edges; returns per-core data + shared K.

    Self-loops are NOT routed through the gather/scatter machinery: their
    contribution (z'_old[d] added to the external aggregate) is folded
    into the on-device combine. They still count toward the degrees.
    """
    row = edge_index[0].astype(np.int64)
    col = edge_index[1].astype(np.int64)
    w = edge_weight.astype(np.float32)

    # degrees exactly as the reference: deg = segment_sum(w, row) with
    # self-loops of weight 1 appended
    deg = np.bincount(row, weights=w.astype(np.float64), minlength=N)
    deg = (deg + 1.0).astype(np.float32)
    dinv = np.where(deg > 0, 1.0 / np.sqrt(np.maximum(deg, 1e-30)), 0.0).astype(
        np.float32
    )

    perm = _make_perm(row, col)
    return _prep_graph2(row, col, w, dinv, perm)


def _make_perm(row, col):
    """slot = perm[core][local_old].

    The chunk schedule pads each (srccore, block) edge count to the
    max-over-cores ceil(cnt/128), so pack each dest core's 12500 nodes
    into its 98 blocks minimizing sum_b max_a ceil(cnt_ab/128): greedy
    over nodes in decreasing max-component in-degree, assigning to the
    bin with the smallest (new K, new max count).
    """
    csrc = row // DPC
    dcnt = np.bincount(col * NCORES + csrc, minlength=N * NCORES).reshape(
        N, NCORES
    )  # per-node in-degree split by source core (incl self-loop)
    perm = np.empty((NCORES, DPC), dtype=np.int64)
    for c in range(NCORES):
        deg = dcnt[c * DPC : (c + 1) * DPC].astype(np.int64)  # [DPC, 8]
        order = np.argsort(-deg.max(axis=1), kind="stable")
        loads = np.zeros((NBLK, NCORES), dtype=np.int64)
        fill = np.zeros(NBLK, dtype=np.int64)
        rank = np.empty(DPC, dtype=np.int64)
        binof = np.empty(DPC, dtype=np.int64)
        for i in order:
            nm = (loads + deg[i]).max(axis=1)
            score = ((nm + 127) >> 7) * 100000 + nm
            score[fill >= 128] = 1 << 60
            b = int(np.argmin(score))
            binof[i] = b
            rank[i] = fill[b]
            fill[b] += 1
            loads[b] += deg[i]
        perm[c] = binof * 128 + rank
    return perm


def _prep_graph2(row, col, w, dinv, perm):
    csrc = row // DPC
    sidx_all = perm[csrc, row - csrc * DPC]  # gather idx in own shard
    assert sidx_all.max() < 32768

    cdst = col // DPC
    ldst = perm[cdst, col - cdst * DPC]
    gb = cdst * NBLK + ldst // 128  # global dest block
    prt = ldst % 128

    # per-(srccore, globalblock) counts -> shared K table
    key = csrc * GBLK + gb
    cnt = np.bincount(key, minlength=NCORES * GBLK).reshape(NCORES, GBLK)
    K = np.maximum(1, (cnt.max(axis=0) + 127) // 128).astype(np.int64)  # [GBLK]

    chunk_off = np.zeros(GBLK, dtype=np.int64)
    chunk_off[1:] = np.cumsum(K)[:-1]
    totch = int(K.sum())
    nslots = totch * 128

    # gather instructions: flat split of the chunk stream
    instr_C = []
    left = totch
    while left > 0:
        c = min(GMAX, left)
        instr_C.append(c)
        left -= c
    instr_C = np.array(instr_C, dtype=np.int64)

    # per-core slot arrays
    per_core = []
    for c in range(NCORES):
        m = csrc == c
        eb, ep, esi, ew = gb[m], prt[m], sidx_all[m], w[m]
        order = np.lexsort((ep, eb))
        eb, ep, esi, ew = eb[order], ep[order], esi[order], ew[order]
        gstart = np.searchsorted(eb, np.arange(GBLK))
        rank = np.arange(len(eb)) - gstart[eb]
        slots = chunk_off[eb] * 128 + rank

        sidx = np.zeros(nslots, dtype=np.int16)   # gather index (pad -> 0)
        sdst = np.full(nslots, 999.0, dtype=np.float32)  # S value (pad -> 999)
        sw = np.zeros(nslots, dtype=np.float32)
        sidx[slots] = esi.astype(np.int16)
        sdst[slots] = ep.astype(np.float32)
        sw[slots] = ew
        per_core.append((sidx, sdst, sw))

    allones = bool(np.all(w == 1.0))
    return dinv, K, chunk_off, instr_C, totch, per_core, perm, allones


def _pack_gidx(sidx, instr_C):
    """Pack int16 gather indices into [16, totch*8] (SWDGE wrap layout).

    Index i of instruction j (chunk offset coff) lands at
    [i%16 + 16*k, coff*8 + i//16] for k in 0..8.
    """
    totch = len(sidx) // 128
    out = np.zeros((16, totch * 8), dtype=np.int16)
    pos = 0
    coff = 0
    for c in instr_C:
        c = int(c)
        n = c * 128
        vals = sidx[pos : pos + n]
        i = np.arange(n)
        out[i % 16, coff * 8 + (i // 16)] = vals
        pos += n
        coff += c
    assert pos == len(sidx)
    return np.tile(out, (8, 1))


def _raw_gather(nc, out_ap, in_ap, idxs_ap, num_idxs, elem_size, stride_bytes):
    """Emit InstDMAGatherAnt directly: the bass helper's 256B elem-size
    assert is a transpose-mode hardware restriction; non-transpose SWDGE
    gathers take byte-granular payloads (mirrored by the executor)."""
    g = nc.gpsimd
    _in_ap = g.lower_ap_dma(in_ap, for_custom_bir_dma=True)
    return g.add_instruction(
        mybir.InstDMAGatherAnt(
            name=g.bass.get_next_instruction_name(),
            ins=[
                *_in_ap,
                g.lower_ap(idxs_ap),
                g.lower_val_access(g.to_reg(num_idxs)),
            ],
            outs=[g.lower_ap(out_ap)],
            transpose=False,
            num_idxs=num_idxs,
            elem_size=elem_size,
            stride_bytes_256=stride_bytes // 256,
            gen_mode=0,
            single_packet=False,
            queue_num=0,
            sbuf_tokens_per_rank=0,
            sbuf_free_dim_per_rank=0,
            sbuf_free_dim_pad_per_rank=0,
            sbuf_byte_offset=0,
        )
    )


def _build_program(K, chunk_off, instr_C, totch, allones=True):
    """Build the SPMD bass program (same for all cores)."""
    nc = bacc.Bacc("TRN2", target_bir_lowering=False, debug=False, num_devices=NCORES)
    RDT = F32 if PF32 else PDT  # partials / ReduceScatter dtype

    # ---- I/O ----
    xT = nc.dram_tensor("xT", [F_IN, SLOTS], PDT, kind="ExternalInput")
    W0c = nc.dram_tensor("W0c", [4, 128, H], PDT, kind="ExternalInput")
    W1 = nc.dram_tensor("W1", [H, H], PDT, kind="ExternalInput")
    b0c = nc.dram_tensor("b0c", [H, 1], F32, kind="ExternalInput")
    b1r = nc.dram_tensor("b1r", [128, H], F32, kind="ExternalInput")
    # coef rows: 0=c2 (0.9*dinv^2), 1=c2L (0.9*dinv), 2=a1 (0.1*dinv), 3=dinv
    coef = nc.dram_tensor("coef", [128, 4, NBLK], F32, kind="ExternalInput")
    iota_d = nc.dram_tensor("iota", [128, 128], PDT, kind="ExternalInput")
    gidx_d = nc.dram_tensor("gidx", [128, totch * 8], I16, kind="ExternalInput")
    destv_d = nc.dram_tensor("destv", [128, totch], F32, kind="ExternalInput")
    ndestv_d = nc.dram_tensor("ndestv", [128, totch], F32, kind="ExternalInput")
    if not allones:
        wv_d = nc.dram_tensor("wv", [128, totch], F32, kind="ExternalInput")
    zout = nc.dram_tensor("zout", [SLOTS, H], F32, kind="ExternalOutput")

    # internal DRAM (double buffered): padded z' shard, partial aggs, RS out
    zp = [nc.dram_tensor(f"zp{i}", [SLOTS, ZPAD], PDT) for i in range(2)]
    part = [nc.dram_tensor(f"part{i}", [128, GBLK, H], RDT) for i in range(2)]
    zr = [nc.dram_tensor(f"zr{i}", [SLOTS, H], RDT) for i in range(2)]

    n_instr = len(instr_C)
    # chunk -> (instr, local offset)
    ch2gi = np.zeros(totch, dtype=np.int64)
    ch2lc = np.zeros(totch, dtype=np.int64)
    instr_coff = np.zeros(n_instr, dtype=np.int64)
    pos = 0
    for gi, c in enumerate(instr_C):
        instr_coff[gi] = pos
        ch2gi[pos : pos + c] = gi
        ch2lc[pos : pos + c] = np.arange(c)
        pos += int(c)

    with tile.TileContext(nc) as tc:
        with (
            tc.tile_pool(name="res", bufs=1) as res,
            tc.tile_pool(name="msg", bufs=4) as msgp,
            tc.tile_pool(name="sp", bufs=12) as sp,
            tc.tile_pool(name="outp", bufs=4) as outp,
            tc.tile_pool(name="psum", bufs=4, space="PSUM") as psp,
        ):
            # ---- residents ----
            iota_sb = res.tile([128, 128], PDT)
            nc.sync.dma_start(out=iota_sb[:], in_=iota_d[:])
            ndestv_sb = res.tile([128, totch], F32)
            nc.sync.dma_start(out=ndestv_sb[:], in_=ndestv_d[:])
            if not allones:
                wv_sb = res.tile([128, totch], F32)
                nc.sync.dma_start(out=wv_sb[:], in_=wv_d[:])
            destv_sb = res.tile([128, totch], F32)
            nc.sync.dma_start(out=destv_sb[:], in_=destv_d[:])
            coef_sb = res.tile([128, 4, NBLK], F32)
            nc.sync.dma_start(out=coef_sb[:], in_=coef[:])
            c2_sb = coef_sb[:, 0, :]
            c2L_sb = coef_sb[:, 1, :]
            a1_sb = coef_sb[:, 2, :]
            dinv_sb = coef_sb[:, 3, :]
            ahd_sb = res.tile([128, NBLK, H], PDT)  # 0.1*dinv*h
            ahL_sb = res.tile([128, NBLK, H], PDT)  # 0.1*h
            w0_sb = res.tile([128, 4, H], PDT)
            nc.sync.dma_start(out=w0_sb[:], in_=W0c.ap().rearrange("k p h -> p k h"))
            w1_sb = res.tile([H, H], PDT)
            nc.sync.dma_start(out=w1_sb[:], in_=W1[:])
            b0_sb = res.tile([H, 1], F32)
            nc.sync.dma_start(out=b0_sb[:], in_=b0c[:])
            b1_sb = res.tile([128, H], F32)
            nc.sync.dma_start(out=b1_sb[:], in_=b1r[:])

            # ---- MLP: h = relu(x@W0+b0)@W1 + b1; z'_0 = dinv*h into zp0;
            # ahd = 0.1*dinv*h, ahL = 0.1*h kept resident ----
            xT_r = xT.ap().rearrange("(k p) c -> p k c", p=128)  # [128,4,SLOTS]
            zp0_r = zp[0].ap().rearrange("(b p) c -> p b c", p=128)
            with (
                tc.tile_pool(name="mlp", bufs=3) as mlp,
                tc.tile_pool(name="mpsum", bufs=2, space="PSUM") as mpsum,
            ):
                for msg_ in range(NBLK // SGB):
                    zslab = outp.tile(
                        [128, SGB, H], F32 if NITER == 0 else PDT, tag="zslab0"
                    )
                    for j in range(SGB):
                        b = msg_ * SGB + j
                        xt = mlp.tile([128, 4, 128], PDT, tag="xt")
                        nc.sync.dma_start(
                            out=xt[:], in_=xT_r[:, :, b * 128 : (b + 1) * 128]
                        )
                        ph1 = mpsum.tile([H, 128], F32, tag="ph1")
                        for k in range(4):
                            nc.tensor.matmul(
                                ph1[:],
                                w0_sb[:, k, :],
                                xt[:, k, :],
                                start=(k == 0),
                                stop=(k == 3),
                            )
                        h1T = mlp.tile([H, 128], PDT, tag="h1T")
                        nc.scalar.activation(
                            h1T[:],
                            ph1[:],
                            mybir.ActivationFunctionType.Relu,
                            bias=b0_sb[:, 0:1],
                        )
                        ph2 = mpsum.tile([128, H], F32, tag="ph2")
                        nc.tensor.matmul(ph2[:], h1T[:], w1_sb[:], start=True, stop=True)
                        ht = mlp.tile([128, H], F32, tag="ht")
                        nc.vector.tensor_tensor(
                            ht[:], ph2[:], b1_sb[:], mybir.AluOpType.add
                        )
                        nc.vector.tensor_scalar_mul(
                            ahd_sb[:, b, :], ht[:], a1_sb[:, b : b + 1]
                        )
                        nc.vector.tensor_scalar_mul(ahL_sb[:, b, :], ht[:], ALPHA)
                        nc.vector.tensor_scalar_mul(
                            zslab[:, j, :], ht[:], dinv_sb[:, b : b + 1]
                        )
                    if NITER == 0:
                        nc.sync.dma_start(
                            out=zout.ap().rearrange("(b p) h -> p b h", p=128)[
                                :, msg_ * SGB : (msg_ + 1) * SGB, :
                            ],
                            in_=zslab[:],
                        )
                    else:
                        nc.sync.dma_start(
                            out=zp0_r[:, msg_ * SGB : (msg_ + 1) * SGB, 0:H],
                            in_=zslab[:],
                        )

            # ---- propagation iterations ----
            for it in range(NITER):
                last = it == NITER - 1
                zsrc = zp[it % 2].ap()[:, 0:H]  # 256B-strided bf16 rows
                tiles = {}  # gi -> msg tile

                def chunk_mt(t, tiles=tiles, zsrc=zsrc):
                    gi = int(ch2gi[t])
                    if gi not in tiles:
                        C = int(instr_C[gi])
                        coff = int(instr_coff[gi])
                        gx = sp.tile([128, GMAX * 8], I16, tag="gx", bufs=3)
                        nc.sync.dma_start(
                            out=gx[:, : C * 8],
                            in_=gidx_d[:, coff * 8 : (coff + C) * 8],
                        )
                        mt = msgp.tile([128, GMAX, H], PDT, tag="msg")
                        if SKIP != "gather":
                            _raw_gather(
                                nc,
                                mt[:, :C, :],
                                zsrc,
                                gx[:, : C * 8],
                                C * 128,
                                H,
                                ZPAD * 2,
                            )
                        else:
                            nc.vector.memset(mt[:, 0:1, :], 0.0)
                        tiles[gi] = mt
                    return tiles[gi][:, int(ch2lc[t]), :]

                for sg in range(NSG):
                    acc = psp.tile([128, SGB * H], F32, name="acc", tag="acc")
                    for j in range(SGB):
                        if SKIP == "mm":
                            break
                        b = sg * SGB + j
                        a = acc[:, j * H : (j + 1) * H]
                        kb = int(K[b])
                        for ck in range(kb):
                            t = int(chunk_off[b]) + ck
                            mtv = chunk_mt(t)
                            if not allones:
                                nc.vector.tensor_scalar_mul(
                                    mtv, mtv, wv_sb[:, t : t + 1]
                                )
                            st = sp.tile([128, 128], PDT, tag="S")
                            r10 = t % 10
                            if r10 < ACT_FRAC10:
                                nc.scalar.activation(
                                    st[:],
                                    iota_sb[:],
                                    mybir.ActivationFunctionType.Abs,
                                    bias=ndestv_sb[:, t : t + 1],
                                )
                                nc.scalar.activation(
                                    st[:],
                                    st[:],
                                    mybir.ActivationFunctionType.Relu,
                                    bias=1.0,
                                    scale=-1.0,
                                )
                            else:
                                seng = (
                                    nc.gpsimd
                                    if r10 < ACT_FRAC10 + POOL_FRAC10
                                    else nc.vector
                                )
                                seng.tensor_scalar(
                                    st[:],
                                    iota_sb[:],
                                    destv_sb[:, t : t + 1],
                                    None,
                                    mybir.AluOpType.is_equal,
                                )
                            nc.tensor.matmul(
                                a,
                                st[:],
                                mtv,
                                start=(ck == 0),
                                stop=(ck == kb - 1),
                            )
                    # drain supergroup PSUM -> partials (one copy + DMA)
                    pslab = outp.tile([128, SGB * H], RDT, tag="pslab")
                    if SKIP == "mm":
                        nc.vector.memset(pslab[:, 0:1], 0.0)
                    else:
                        nc.vector.tensor_copy(pslab[:], acc[:])
                    nc.sync.dma_start(
                        out=part[it % 2].ap()[:, sg * SGB : (sg + 1) * SGB, :],
                        in_=pslab[:].rearrange("p (b h) -> p b h", h=H),
                    )

                # ReduceScatter over the global partials; core c receives
                # its own 98 blocks (transposed AP: (b p) iteration order)
                nc.gpsimd.collective_compute(
                    "ReduceScatter",
                    mybir.AluOpType.add,
                    replica_groups=[list(range(NCORES))],
                    ins=[part[it % 2].ap().rearrange("p b h -> b p h").opt()],
                    outs=[zr[it % 2].ap().opt()],
                )

                # combine (self-loop folded in): agg = zr + z'_old;
                #   non-last: z' = c2*agg + ahd (bf16)
                #   last:     z  = c2L*agg + ahL (f32)
                # as 2 fused ops: t1 = (zr*c)+ah; out = (z'_old*c)+t1
                zr_r = zr[it % 2].ap().rearrange("(b p) h -> p b h", p=128)
                zold_r = zp[it % 2].ap().rearrange("(b p) c -> p b c", p=128)
                cmul = c2L_sb if last else c2_sb
                ah = ahL_sb if last else ahd_sb
                zdst_r = (
                    zout.ap().rearrange("(b p) h -> p b h", p=128)
                    if last
                    else zp[(it + 1) % 2].ap().rearrange("(b p) c -> p b c", p=128)
                )
                for cg in range(NBLK // SGB):
                    zrt = outp.tile([128, SGB, H], RDT, tag="zrt")
                    nc.sync.dma_start(
                        out=zrt[:], in_=zr_r[:, cg * SGB : (cg + 1) * SGB, :]
                    )
                    zot = outp.tile([128, SGB, H], PDT, tag="zot")
                    nc.sync.dma_start(
                        out=zot[:], in_=zold_r[:, cg * SGB : (cg + 1) * SGB, 0:H]
                    )
                    zslab = outp.tile(
                        [128, SGB, H], F32 if last else PDT,
                        tag="zslabL" if last else "zslab",
                    )
                    for j in range(SGB):
                        b = cg * SGB + j
                        tmp = outp.tile([128, H], F32, tag="ctmp")
                        nc.vector.scalar_tensor_tensor(
                            tmp[:],
                            zrt[:, j, :],
                            cmul[:, b : b + 1],
                            ah[:, b, :],
                            mybir.AluOpType.mult,
                            mybir.AluOpType.add,
                        )
                        nc.vector.scalar_tensor_tensor(
                            zslab[:, j, :],
                            zot[:, j, :],
                            cmul[:, b : b + 1],
                            tmp[:],
                            mybir.AluOpType.mult,
                            mybir.AluOpType.add,
                        )
                    if last:
                        nc.sync.dma_start(
                            out=zdst_r[:, cg * SGB : (cg + 1) * SGB, :],
                            in_=zslab[:],
                        )
                    else:
                        nc.sync.dma_start(
                            out=zdst_r[:, cg * SGB : (cg + 1) * SGB, 0:H],
                            in_=zslab[:],
                        )

    nc.compile()
    return nc


def kernel(x, edge_index, edge_weight, W0, b0, W1, b1):
    x = np.asarray(x, dtype=np.float32)
    dinv, K, chunk_off, instr_C, totch, per_core, perm, allones = _prep_graph(
        np.asarray(edge_index), np.asarray(edge_weight)
    )

    in_maps = []
    for c in range(NCORES):
        sidx, sdst, sw = per_core[c]
        g = _pack_gidx(sidx, instr_C)

        destv = sdst.reshape(totch, 128).T.copy()  # [128, totch]

        xs = np.zeros((SLOTS, F_IN), dtype=np.float32)
        xs[perm[c]] = x[c * DPC : (c + 1) * DPC]
        xT = np.ascontiguousarray(xs.T).astype(NPPDT)  # [F_IN, SLOTS]

        dv = np.zeros(SLOTS, dtype=np.float32)
        dv[perm[c]] = dinv[c * DPC : (c + 1) * DPC]
        dv2 = dv.reshape(NBLK, 128).T  # [128, NBLK]
        coef = np.ascontiguousarray(
            np.stack(
                [
                    (1.0 - ALPHA) * dv2 * dv2,  # c2
                    (1.0 - ALPHA) * dv2,        # c2L
                    ALPHA * dv2,                # a1
                    dv2,                        # dinv
                ]
            ).transpose(1, 0, 2)
        ).astype(np.float32)

        in_maps.append(
            {
                "xT": xT,
                "W0c": np.asarray(W0, np.float32).reshape(4, 128, H).astype(NPPDT),
                "W1": np.asarray(W1, np.float32).astype(NPPDT),
                "b0c": np.asarray(b0, np.float32).reshape(H, 1).copy(),
                "b1r": np.broadcast_to(
                    np.asarray(b1, np.float32), (128, H)
                ).copy(),
                "coef": coef,
                "iota": np.broadcast_to(
                    np.arange(128, dtype=np.float32), (128, 128)
                ).astype(NPPDT),
                "gidx": g,
                "destv": destv,
                "ndestv": -destv,
                **(
                    {}
                    if allones
                    else {"wv": sw.reshape(totch, 128).T.copy()}
                ),
            }
        )

    nc = _build_program(K, chunk_off, instr_C, totch, allones)
    res = run_bass_kernel_spmd(nc, in_maps, core_ids=list(range(NCORES)))

    global LAST_PERM, LAST_NC
    LAST_PERM = perm
    LAST_NC = nc
    out = np.empty((N, H), dtype=np.float32)
    for c in range(NCORES):
        out[c * DPC : (c + 1) * DPC] = res.results[c]["zout"][perm[c]]
    return out


# revision 23
# speedup vs baseline: 1.2542x; 1.0377x over previous
"""APPNP (MLP + 10 sparse propagation iterations) on 8 Trainium2 NeuronCores.

Design (source-sharded; all FLOPs on device, host does indexing only):
  - Nodes sharded by id: core c owns nodes [c*12500, (c+1)*12500) as BOTH
    source shard (z' rows it gathers from) and dest shard (the 98 local
    blocks it combines after the ReduceScatter). Per-core local slot
    layout from a serpentine in-degree bin-pack (98 blocks x 128 slots).
  - Edges partitioned by SOURCE core. Each core gathers its edges' source
    rows from its OWN z' shard only (no all-gather): z' lives bf16 in
    256B-strided padded rows ([12544, 128] bf16, data in cols 0:64) so the
    SWDGE gather uses 128B-payload descriptors (the 256B-alignment assert
    in bass.dma_gather is a transpose-only hardware restriction; the
    instruction is emitted directly with elem_size=64/elem_step=128).
  - Scatter-add over the GLOBAL dest space (784 blocks = 8 cores x 98) as
    one-hot selection-matrix matmuls into per-supergroup PSUM f32
    accumulators (S built on-device in bf16: DVE is_equal runs in 4x mode;
    a configurable fraction on ScalarE as Abs/Relu pairs). Chunk schedule
    shared across cores via a max-over-cores K table; chunks stream in
    <=63-chunk gather instructions consumed in emission order.
  - Per iteration ONE ReduceScatter(add) reduces the f32 partial
    aggregations [128, 784, 64] (partition-major for contiguous 1.8KB
    write descriptors; the collective reads a transposed AP so each core
    receives its own 98 blocks) into zr [12544, 64] f32.
  - Combine (2 fused DVE ops per block): z' = 0.9*dinv^2*agg + 0.1*dinv*h
    (bf16, into padded zp rows); last iteration z = 0.9*dinv*agg + 0.1*h
    written f32 to zout.
  - MLP (h = relu(x@W0+b0)@W1+b1) runs once on-device in bf16 from a
    host-transposed x shard; precomputes ahd=0.1*dinv*h and ahL=0.1*h.
"""

import os
import numpy as np
import ml_dtypes

import concourse.bass as bass
import concourse.bacc as bacc
import concourse.tile as tile
import concourse.mybir as mybir
from concourse.bass_utils import run_bass_kernel_spmd

F32 = mybir.dt.float32
BF16 = mybir.dt.bfloat16
I16 = mybir.dt.int16
NPBF16 = ml_dtypes.bfloat16

N = 100000
F_IN = 512
H = 64
NCORES = 8
ALPHA = 0.1
NITER = int(os.environ.get("APPNP_NITER", "10"))
SKIP = os.environ.get("APPNP_SKIP", "")
ACT_FRAC10 = int(os.environ.get("APPNP_ACT10", "1"))
POOL_FRAC10 = int(os.environ.get("APPNP_POOL10", "0"))
PF32 = bool(os.environ.get("APPNP_PF32", "1"))  # f32 partials+ReduceScatter

DPC = N // NCORES          # 12500 real nodes per core
NBLK = 98                  # local blocks of 128 dest slots
SLOTS = NBLK * 128         # 12544 padded slots per core
GBLK = NCORES * NBLK       # 784 global dest blocks
SGB = 7                    # blocks per supergroup
NSG = GBLK // SGB          # 112 supergroups (global)
NTOT = NCORES * SLOTS      # 100352 global dest slots
GMAX = 63                  # chunks per dma_gather instruction
ZPAD = 128                 # padded z' row width (256B bf16 stride)

PDT = BF16
NPPDT = NPBF16


def _prep_graph(edge_index, edge_weight):
    """Host-side: shard/sort/pad edges; returns per-core data + shared K.

    Self-loops are NOT routed through the gather/scatter machinery: their
    contribution (z'_old[d] added to the external aggregate) is folded
    into the on-device combine. They still count toward the degrees.
    """
    row = edge_index[0].astype(np.int64)
    col = edge_index[1].astype(np.int64)
    w = edge_weight.astype(np.float32)

    # degrees exactly as the reference: deg = segment_sum(w, row) with
    # self-loops of weight 1 appended
    deg = np.bincount(row, weights=w.astype(np.float64), minlength=N)
    deg = (deg + 1.0).astype(np.float32)
    dinv = np.where(deg > 0, 1.0 / np.sqrt(np.maximum(deg, 1e-30)), 0.0).astype(
        np.float32
    )

    perm = _make_perm(row, col)
    return _prep_graph2(row, col, w, dinv, perm)


def _make_perm(row, col):
    """slot = perm[core][local_old].

    The chunk schedule pads each (srccore, block) edge count to the
    max-over-cores ceil(cnt/128), so pack each dest core's 12500 nodes
    into its 98 blocks minimizing sum_b max_a ceil(cnt_ab/128): greedy
    over nodes in decreasing max-component in-degree, assigning to the
    bin with the smallest (new K, new max count).
    """
    csrc = row // DPC
    dcnt = np.bincount(col * NCORES + csrc, minlength=N * NCORES).reshape(
        N, NCORES
    )  # per-node in-degree split by source core (incl self-loop)
    perm = np.empty((NCORES, DPC), dtype=np.int64)
    for c in range(NCORES):
        deg = dcnt[c * DPC : (c + 1) * DPC].astype(np.int64)  # [DPC, 8]
        order = np.argsort(-deg.max(axis=1), kind="stable")
        loads = np.zeros((NBLK, NCORES), dtype=np.int64)
        fill = np.zeros(NBLK, dtype=np.int64)
        rank = np.empty(DPC, dtype=np.int64)
        binof = np.empty(DPC, dtype=np.int64)
        for i in order:
            nm = (loads + deg[i]).max(axis=1)
            score = ((nm + 127) >> 7) * 100000 + nm
            score[fill >= 128] = 1 << 60
            b = int(np.argmin(score))
            binof[i] = b
            rank[i] = fill[b]
            fill[b] += 1
            loads[b] += deg[i]
        _repair(deg, binof, loads)
        rank = np.zeros(DPC, dtype=np.int64)
        fill[:] = 0
        for i in range(DPC):
            rank[i] = fill[binof[i]]
            fill[binof[i]] += 1
        perm[c] = binof * 128 + rank
    return perm


def _repair(deg, binof, loads):
    """Swap nodes across bins to drop just-over-boundary blocks to a
    smaller chunk count K (every saved chunk = 128 fewer gather
    descriptors + one fewer S-build + matmul per iteration)."""
    members = [np.where(binof == b)[0] for b in range(NBLK)]
    for _ in range(4):
        K = (loads.max(axis=1) + 127) // 128
        improved = 0
        for b in np.argsort(loads.max(axis=1) - (K - 1) * 128):
            bound = (int(K[b]) - 1) * 128
            if bound <= 0 or loads[b].max() <= bound:
                continue
            over = loads[b].max() - bound
            if over > 24:
                continue
            a_star = int(loads[b].argmax())
            mb = members[b]
            u_order = mb[np.argsort(-deg[mb, a_star])][:6]
            done = False
            for u in u_order:
                # candidate destination bins: largest slack under their K
                slack = K * 128 - loads.max(axis=1)
                for b2 in np.argsort(-slack)[:8]:
                    if b2 == b:
                        continue
                    m2 = members[b2]
                    # v light on a_star
                    v = m2[int(np.argmin(deg[m2, a_star]))]
                    nb = loads[b] - deg[u] + deg[v]
                    nb2 = loads[b2] - deg[v] + deg[u]
                    if nb.max() <= bound and nb2.max() <= int(K[b2]) * 128:
                        loads[b] = nb
                        loads[b2] = nb2
                        binof[u], binof[v] = b2, b
                        members[b] = np.append(mb[mb != u], v)
                        members[b2] = np.append(m2[m2 != v], u)
                        improved += 1
                        done = True
                        break
                if done:
                    break
        if not improved:
            break


def _prep_graph2(row, col, w, dinv, perm):
    csrc = row // DPC
    sidx_all = perm[csrc, row - csrc * DPC]  # gather idx in own shard
    assert sidx_all.max() < 32768

    cdst = col // DPC
    ldst = perm[cdst, col - cdst * DPC]
    gb = cdst * NBLK + ldst // 128  # global dest block
    prt = ldst % 128

    # per-(srccore, globalblock) counts -> shared K table
    key = csrc * GBLK + gb
    cnt = np.bincount(key, minlength=NCORES * GBLK).reshape(NCORES, GBLK)
    K = np.maximum(1, (cnt.max(axis=0) + 127) // 128).astype(np.int64)  # [GBLK]

    chunk_off = np.zeros(GBLK, dtype=np.int64)
    chunk_off[1:] = np.cumsum(K)[:-1]
    totch = int(K.sum())
    nslots = totch * 128

    # gather instructions: flat split of the chunk stream
    instr_C = []
    left = totch
    while left > 0:
        c = min(GMAX, left)
        instr_C.append(c)
        left -= c
    instr_C = np.array(instr_C, dtype=np.int64)

    # per-core slot arrays
    per_core = []
    for c in range(NCORES):
        m = csrc == c
        eb, ep, esi, ew = gb[m], prt[m], sidx_all[m], w[m]
        order = np.lexsort((ep, eb))
        eb, ep, esi, ew = eb[order], ep[order], esi[order], ew[order]
        gstart = np.searchsorted(eb, np.arange(GBLK))
        rank = np.arange(len(eb)) - gstart[eb]
        slots = chunk_off[eb] * 128 + rank

        sidx = np.zeros(nslots, dtype=np.int16)   # gather index (pad -> 0)
        sdst = np.full(nslots, 999.0, dtype=np.float32)  # S value (pad -> 999)
        sw = np.zeros(nslots, dtype=np.float32)
        sidx[slots] = esi.astype(np.int16)
        sdst[slots] = ep.astype(np.float32)
        sw[slots] = ew
        per_core.append((sidx, sdst, sw))

    allones = bool(np.all(w == 1.0))
    return dinv, K, chunk_off, instr_C, totch, per_core, perm, allones


def _pack_gidx(sidx, instr_C):
    """Pack int16 gather indices into [16, totch*8] (SWDGE wrap layout).

    Index i of instruction j (chunk offset coff) lands at
    [i%16 + 16*k, coff*8 + i//16] for k in 0..8.
    """
    totch = len(sidx) // 128
    out = np.zeros((16, totch * 8), dtype=np.int16)
    pos = 0
    coff = 0
    for c in instr_C:
        c = int(c)
        n = c * 128
        vals = sidx[pos : pos + n]
        i = np.arange(n)
        out[i % 16, coff * 8 + (i // 16)] = vals
        pos += n
        coff += c
    assert pos == len(sidx)
    return np.tile(out, (8, 1))


def _raw_gather(nc, out_ap, in_ap, idxs_ap, num_idxs, elem_size, stride_bytes):
    """Emit InstDMAGatherAnt directly: the bass helper's 256B elem-size
    assert is a transpose-mode hardware restriction; non-transpose SWDGE
    gathers take byte-granular payloads (mirrored by the executor)."""
    g = nc.gpsimd
    _in_ap = g.lower_ap_dma(in_ap, for_custom_bir_dma=True)
    return g.add_instruction(
        mybir.InstDMAGatherAnt(
            name=g.bass.get_next_instruction_name(),
            ins=[
                *_in_ap,
                g.lower_ap(idxs_ap),
                g.lower_val_access(g.to_reg(num_idxs)),
            ],
            outs=[g.lower_ap(out_ap)],
            transpose=False,
            num_idxs=num_idxs,
            elem_size=elem_size,
            stride_bytes_256=stride_bytes // 256,
            gen_mode=0,
            single_packet=False,
            queue_num=0,
            sbuf_tokens_per_rank=0,
            sbuf_free_dim_per_rank=0,
            sbuf_free_dim_pad_per_rank=0,
            sbuf_byte_offset=0,
        )
    )


def _build_program(K, chunk_off, instr_C, totch, allones=True):
    """Build the SPMD bass program (same for all cores)."""
    nc = bacc.Bacc("TRN2", target_bir_lowering=False, debug=False, num_devices=NCORES)
    RDT = F32 if PF32 else PDT  # partials / ReduceScatter dtype

    # ---- I/O ----
    xT = nc.dram_tensor("xT", [F_IN, SLOTS], PDT, kind="ExternalInput")
    W0c = nc.dram_tensor("W0c", [4, 128, H], PDT, kind="ExternalInput")
    W1 = nc.dram_tensor("W1", [H, H], PDT, kind="ExternalInput")
    b0c = nc.dram_tensor("b0c", [H, 1], F32, kind="ExternalInput")
    b1r = nc.dram_tensor("b1r", [128, H], F32, kind="ExternalInput")
    # coef rows: 0=c2 (0.9*dinv^2), 1=c2L (0.9*dinv), 2=a1 (0.1*dinv), 3=dinv
    coef = nc.dram_tensor("coef", [128, 4, NBLK], F32, kind="ExternalInput")
    iota_d = nc.dram_tensor("iota", [128, 128], PDT, kind="ExternalInput")
    gidx_d = nc.dram_tensor("gidx", [128, totch * 8], I16, kind="ExternalInput")
    destv_d = nc.dram_tensor("destv", [128, totch], F32, kind="ExternalInput")
    ndestv_d = nc.dram_tensor("ndestv", [128, totch], F32, kind="ExternalInput")
    if not allones:
        wv_d = nc.dram_tensor("wv", [128, totch], F32, kind="ExternalInput")
    zout = nc.dram_tensor("zout", [SLOTS, H], F32, kind="ExternalOutput")

    # internal DRAM (double buffered): padded z' shard, partial aggs, RS out.
    # part layout: [destcore, partition, localblock, h] — each core's RS
    # section is partition-major so drain writes and the collective input
    # are contiguous (the BIR verifier rejects strided collective APs).
    zp = [nc.dram_tensor(f"zp{i}", [SLOTS, ZPAD], PDT) for i in range(2)]
    part = [
        nc.dram_tensor(f"part{i}", [NCORES, 128, NBLK, H], RDT) for i in range(2)
    ]
    zr = [nc.dram_tensor(f"zr{i}", [128, NBLK, H], RDT) for i in range(2)]

    n_instr = len(instr_C)
    # chunk -> (instr, local offset)
    ch2gi = np.zeros(totch, dtype=np.int64)
    ch2lc = np.zeros(totch, dtype=np.int64)
    instr_coff = np.zeros(n_instr, dtype=np.int64)
    pos = 0
    for gi, c in enumerate(instr_C):
        instr_coff[gi] = pos
        ch2gi[pos : pos + c] = gi
        ch2lc[pos : pos + c] = np.arange(c)
        pos += int(c)

    with tile.TileContext(nc) as tc:
        with (
            tc.tile_pool(name="res", bufs=1) as res,
            tc.tile_pool(name="msg", bufs=4) as msgp,
            tc.tile_pool(name="sp", bufs=12) as sp,
            tc.tile_pool(name="outp", bufs=4) as outp,
            tc.tile_pool(name="psum", bufs=4, space="PSUM") as psp,
        ):
            # ---- residents ----
            iota_sb = res.tile([128, 128], PDT)
            nc.sync.dma_start(out=iota_sb[:], in_=iota_d[:])
            ndestv_sb = res.tile([128, totch], F32)
            nc.sync.dma_start(out=ndestv_sb[:], in_=ndestv_d[:])
            if not allones:
                wv_sb = res.tile([128, totch], F32)
                nc.sync.dma_start(out=wv_sb[:], in_=wv_d[:])
            destv_sb = res.tile([128, totch], F32)
            nc.sync.dma_start(out=destv_sb[:], in_=destv_d[:])
            coef_sb = res.tile([128, 4, NBLK], F32)
            nc.sync.dma_start(out=coef_sb[:], in_=coef[:])
            c2_sb = coef_sb[:, 0, :]
            c2L_sb = coef_sb[:, 1, :]
            a1_sb = coef_sb[:, 2, :]
            dinv_sb = coef_sb[:, 3, :]
            ahd_sb = res.tile([128, NBLK, H], PDT)  # 0.1*dinv*h
            ahL_sb = res.tile([128, NBLK, H], PDT)  # 0.1*h
            w0_sb = res.tile([128, 4, H], PDT)
            nc.sync.dma_start(out=w0_sb[:], in_=W0c.ap().rearrange("k p h -> p k h"))
            w1_sb = res.tile([H, H], PDT)
            nc.sync.dma_start(out=w1_sb[:], in_=W1[:])
            b0_sb = res.tile([H, 1], F32)
            nc.sync.dma_start(out=b0_sb[:], in_=b0c[:])
            b1_sb = res.tile([128, H], F32)
            nc.sync.dma_start(out=b1_sb[:], in_=b1r[:])

            # ---- MLP: h = relu(x@W0+b0)@W1 + b1; z'_0 = dinv*h into zp0;
            # ahd = 0.1*dinv*h, ahL = 0.1*h kept resident ----
            xT_r = xT.ap().rearrange("(k p) c -> p k c", p=128)  # [128,4,SLOTS]
            zp0_r = zp[0].ap().rearrange("(b p) c -> p b c", p=128)
            with (
                tc.tile_pool(name="mlp", bufs=3) as mlp,
                tc.tile_pool(name="mpsum", bufs=2, space="PSUM") as mpsum,
            ):
                for msg_ in range(NBLK // SGB):
                    zslab = outp.tile(
                        [128, SGB, H], F32 if NITER == 0 else PDT, tag="zslab0"
                    )
                    for j in range(SGB):
                        b = msg_ * SGB + j
                        xt = mlp.tile([128, 4, 128], PDT, tag="xt")
                        nc.sync.dma_start(
                            out=xt[:], in_=xT_r[:, :, b * 128 : (b + 1) * 128]
                        )
                        ph1 = mpsum.tile([H, 128], F32, tag="ph1")
                        for k in range(4):
                            nc.tensor.matmul(
                                ph1[:],
                                w0_sb[:, k, :],
                                xt[:, k, :],
                                start=(k == 0),
                                stop=(k == 3),
                            )
                        h1T = mlp.tile([H, 128], PDT, tag="h1T")
                        nc.scalar.activation(
                            h1T[:],
                            ph1[:],
                            mybir.ActivationFunctionType.Relu,
                            bias=b0_sb[:, 0:1],
                        )
                        ph2 = mpsum.tile([128, H], F32, tag="ph2")
                        nc.tensor.matmul(ph2[:], h1T[:], w1_sb[:], start=True, stop=True)
                        ht = mlp.tile([128, H], F32, tag="ht")
                        nc.vector.tensor_tensor(
                            ht[:], ph2[:], b1_sb[:], mybir.AluOpType.add
                        )
                        nc.vector.tensor_scalar_mul(
                            ahd_sb[:, b, :], ht[:], a1_sb[:, b : b + 1]
                        )
                        nc.vector.tensor_scalar_mul(ahL_sb[:, b, :], ht[:], ALPHA)
                        nc.vector.tensor_scalar_mul(
                            zslab[:, j, :], ht[:], dinv_sb[:, b : b + 1]
                        )
                    if NITER == 0:
                        nc.sync.dma_start(
                            out=zout.ap().rearrange("(b p) h -> p b h", p=128)[
                                :, msg_ * SGB : (msg_ + 1) * SGB, :
                            ],
                            in_=zslab[:],
                        )
                    else:
                        nc.sync.dma_start(
                            out=zp0_r[:, msg_ * SGB : (msg_ + 1) * SGB, 0:H],
                            in_=zslab[:],
                        )

            # ---- propagation iterations ----
            for it in range(NITER):
                last = it == NITER - 1
                zsrc = zp[it % 2].ap()[:, 0:H]  # 256B-strided bf16 rows
                tiles = {}  # gi -> msg tile

                def chunk_mt(t, tiles=tiles, zsrc=zsrc):
                    gi = int(ch2gi[t])
                    if gi not in tiles:
                        C = int(instr_C[gi])
                        coff = int(instr_coff[gi])
                        gx = sp.tile([128, GMAX * 8], I16, tag="gx", bufs=3)
                        nc.sync.dma_start(
                            out=gx[:, : C * 8],
                            in_=gidx_d[:, coff * 8 : (coff + C) * 8],
                        )
                        mt = msgp.tile([128, GMAX, H], PDT, tag="msg")
                        if SKIP != "gather":
                            _raw_gather(
                                nc,
                                mt[:, :C, :],
                                zsrc,
                                gx[:, : C * 8],
                                C * 128,
                                H,
                                ZPAD * 2,
                            )
                        else:
                            nc.vector.memset(mt[:, 0:1, :], 0.0)
                        tiles[gi] = mt
                    return tiles[gi][:, int(ch2lc[t]), :]

                for sg in range(NSG):
                    acc = psp.tile([128, SGB * H], F32, name="acc", tag="acc")
                    for j in range(SGB):
                        if SKIP == "mm":
                            break
                        b = sg * SGB + j
                        a = acc[:, j * H : (j + 1) * H]
                        kb = int(K[b])
                        for ck in range(kb):
                            t = int(chunk_off[b]) + ck
                            mtv = chunk_mt(t)
                            if not allones:
                                nc.vector.tensor_scalar_mul(
                                    mtv, mtv, wv_sb[:, t : t + 1]
                                )
                            st = sp.tile([128, 128], PDT, tag="S")
                            r10 = t % 10
                            if r10 < ACT_FRAC10:
                                nc.scalar.activation(
                                    st[:],
                                    iota_sb[:],
                                    mybir.ActivationFunctionType.Abs,
                                    bias=ndestv_sb[:, t : t + 1],
                                )
                                nc.scalar.activation(
                                    st[:],
                                    st[:],
                                    mybir.ActivationFunctionType.Relu,
                                    bias=1.0,
                                    scale=-1.0,
                                )
                            else:
                                seng = (
                                    nc.gpsimd
                                    if r10 < ACT_FRAC10 + POOL_FRAC10
                                    else nc.vector
                                )
                                seng.tensor_scalar(
                                    st[:],
                                    iota_sb[:],
                                    destv_sb[:, t : t + 1],
                                    None,
                                    mybir.AluOpType.is_equal,
                                )
                            nc.tensor.matmul(
                                a,
                                st[:],
                                mtv,
                                start=(ck == 0),
                                stop=(ck == kb - 1),
                            )
                    # drain supergroup PSUM -> partials (one copy + DMA)
                    pslab = outp.tile([128, SGB * H], RDT, tag="pslab")
                    if SKIP == "mm":
                        nc.vector.memset(pslab[:, 0:1], 0.0)
                    else:
                        nc.vector.tensor_copy(pslab[:], acc[:])
                    sgc, sgl = sg // (NBLK // SGB), sg % (NBLK // SGB)
                    nc.sync.dma_start(
                        out=part[it % 2].ap()[
                            sgc, :, sgl * SGB : (sgl + 1) * SGB, :
                        ],
                        in_=pslab[:].rearrange("p (b h) -> p b h", h=H),
                    )

                # ReduceScatter over the global partials; core c receives
                # its own 98 blocks (transposed AP: (b p) iteration order)
                nc.gpsimd.collective_compute(
                    "ReduceScatter",
                    mybir.AluOpType.add,
                    replica_groups=[list(range(NCORES))],
                    ins=[part[it % 2].ap().opt()],
                    outs=[zr[it % 2].ap().opt()],
                )

                # combine (self-loop folded in): agg = zr + z'_old;
                #   non-last: z' = c2*agg + ahd (bf16)
                #   last:     z  = c2L*agg + ahL (f32)
                # as 2 fused ops: t1 = (zr*c)+ah; out = (z'_old*c)+t1
                zr_r = zr[it % 2].ap()  # [128, NBLK, H]
                zold_r = zp[it % 2].ap().rearrange("(b p) c -> p b c", p=128)
                cmul = c2L_sb if last else c2_sb
                ah = ahL_sb if last else ahd_sb
                zdst_r = (
                    zout.ap().rearrange("(b p) h -> p b h", p=128)
                    if last
                    else zp[(it + 1) % 2].ap().rearrange("(b p) c -> p b c", p=128)
                )
                for cg in range(NBLK // SGB):
                    zrt = outp.tile([128, SGB, H], RDT, tag="zrt")
                    nc.sync.dma_start(
                        out=zrt[:], in_=zr_r[:, cg * SGB : (cg + 1) * SGB, :]
                    )
                    zot = outp.tile([128, SGB, H], PDT, tag="zot")
                    nc.sync.dma_start(
                        out=zot[:], in_=zold_r[:, cg * SGB : (cg + 1) * SGB, 0:H]
                    )
                    zslab = outp.tile(
                        [128, SGB, H], F32 if last else PDT,
                        tag="zslabL" if last else "zslab",
                    )
                    for j in range(SGB):
                        b = cg * SGB + j
                        tmp = outp.tile([128, H], F32, tag="ctmp")
                        nc.vector.scalar_tensor_tensor(
                            tmp[:],
                            zrt[:, j, :],
                            cmul[:, b : b + 1],
                            ah[:, b, :],
                            mybir.AluOpType.mult,
                            mybir.AluOpType.add,
                        )
                        nc.vector.scalar_tensor_tensor(
                            zslab[:, j, :],
                            zot[:, j, :],
                            cmul[:, b : b + 1],
                            tmp[:],
                            mybir.AluOpType.mult,
                            mybir.AluOpType.add,
                        )
                    if last:
                        nc.sync.dma_start(
                            out=zdst_r[:, cg * SGB : (cg + 1) * SGB, :],
                            in_=zslab[:],
                        )
                    else:
                        nc.sync.dma_start(
                            out=zdst_r[:, cg * SGB : (cg + 1) * SGB, 0:H],
                            in_=zslab[:],
                        )

    nc.compile()
    return nc


def kernel(x, edge_index, edge_weight, W0, b0, W1, b1):
    x = np.asarray(x, dtype=np.float32)
    dinv, K, chunk_off, instr_C, totch, per_core, perm, allones = _prep_graph(
        np.asarray(edge_index), np.asarray(edge_weight)
    )

    in_maps = []
    for c in range(NCORES):
        sidx, sdst, sw = per_core[c]
        g = _pack_gidx(sidx, instr_C)

        destv = sdst.reshape(totch, 128).T.copy()  # [128, totch]

        xs = np.zeros((SLOTS, F_IN), dtype=np.float32)
        xs[perm[c]] = x[c * DPC : (c + 1) * DPC]
        xT = np.ascontiguousarray(xs.T).astype(NPPDT)  # [F_IN, SLOTS]

        dv = np.zeros(SLOTS, dtype=np.float32)
        dv[perm[c]] = dinv[c * DPC : (c + 1) * DPC]
        dv2 = dv.reshape(NBLK, 128).T  # [128, NBLK]
        coef = np.ascontiguousarray(
            np.stack(
                [
                    (1.0 - ALPHA) * dv2 * dv2,  # c2
                    (1.0 - ALPHA) * dv2,        # c2L
                    ALPHA * dv2,                # a1
                    dv2,                        # dinv
                ]
            ).transpose(1, 0, 2)
        ).astype(np.float32)

        in_maps.append(
            {
                "xT": xT,
                "W0c": np.asarray(W0, np.float32).reshape(4, 128, H).astype(NPPDT),
                "W1": np.asarray(W1, np.float32).astype(NPPDT),
                "b0c": np.asarray(b0, np.float32).reshape(H, 1).copy(),
                "b1r": np.broadcast_to(
                    np.asarray(b1, np.float32), (128, H)
                ).copy(),
                "coef": coef,
                "iota": np.broadcast_to(
                    np.arange(128, dtype=np.float32), (128, 128)
                ).astype(NPPDT),
                "gidx": g,
                "destv": destv,
                "ndestv": -destv,
                **(
                    {}
                    if allones
                    else {"wv": sw.reshape(totch, 128).T.copy()}
                ),
            }
        )

    nc = _build_program(K, chunk_off, instr_C, totch, allones)
    res = run_bass_kernel_spmd(nc, in_maps, core_ids=list(range(NCORES)))

    global LAST_PERM, LAST_NC
    LAST_PERM = perm
    LAST_NC = nc
    out = np.empty((N, H), dtype=np.float32)
    for c in range(NCORES):
        out[c * DPC : (c + 1) * DPC] = res.results[c]["zout"][perm[c]]
    return out
